# revision 49
# baseline (speedup 1.0000x reference)
"""Trainium2 Bass kernel for CentroidLayer inference.

reference math:
    _, V = eigh(C)                              # [NC, N_CH, P, P]
    diag[b,c,n,i] = sum_{j,k} V[c,n,j,i] * X[b,n,j,k] * V[c,n,k,i]

Strategy:
  * eigh(C) on host (eigenvector sign ambiguity cancels in the bilinear
    form, so any correct eigh matches the reference up to fp rounding).
  * Shard the 8 channels (N_CH) across the 8 NeuronCores — each core
    handles one channel end-to-end; inputs/outputs per core are 1/8 of
    the total, and the centroid data for one channel is tiny (128 KB).
  * Per core, rewrite the contraction as one big matmul:
        out[b, (c,i)] = sum_{jk} X[b, (j,k)] * W[(j,k), (c,i)]
        W[(j,k), (c,i)] = V[c,j,i] * V[c,k,i]
    W ([1024, 1024], 4 MB) is built ON DEVICE from V (128 KB) to keep
    HBM traffic minimal:
        Urep[(jj,k), ci] = U[k, ci]      (U tiled 4x on partitions; DMA'd)
        Ubc [(j,k),  ci] = U[j, ci]      (PE: selection-matrix matmuls)
        W = Ubc * Urep                   (DVE elementwise)
    Main matmuls run in float32r (4x faster than float32 on the PE).
"""

import os
import sys

import numpy as np

B, NC, N_CH, P = 256, 32, 8, 32
CI = NC * P          # 1024 (c,i) pairs
JK = P * P           # 1024 (j,k) pairs
NKC = JK // 128      # 8 contraction chunks of 128
NBH = B // 128       # 2 batch halves of 128

# dtype knobs for the PE (empirically tuned; float32r is the fast fp32 path)
MAIN_F32R = os.environ.get("KERNEL_MAIN_F32R", "1") == "1"
BSEL_F32R = os.environ.get("KERNEL_BSEL_F32R", "1") == "1"

_PROGRAM = None
# sym: host-built symmetric W, raw bass (default / fastest)
# raw: on-device W construction, raw bass
# tile: on-device W construction, Tile framework
MODE = os.environ.get("KERNEL_MODE", "sym")


def _import_concourse():
    try:
        import concourse  # noqa: F401
    except ImportError:
        for p in ("/opt/trn_rl_repo", os.path.expanduser("~/trn_rl_repo")):
            if os.path.isdir(p):
                sys.path.insert(0, p)
                break
        import concourse  # noqa: F401
    _ensure_axon_hooks()


def _ensure_axon_hooks():
    """This image's `antenv` lacks `axon_hooks`; concourse imports it when
    trace=True. Provide the module + register the ctypes NTFF hook so
    profiling works (best-effort; everything still runs without it)."""
    try:
        import antenv.axon_hooks  # noqa: F401

        return
    except ImportError:
        pass
    try:
        import types

        import antenv

        mod = types.ModuleType("antenv.axon_hooks")
        holder = {"hook": None}
        mod.set_axon_ntff_profile_hook = lambda h: holder.__setitem__("hook", h)
        mod.get_axon_ntff_profile_hook = lambda: holder["hook"]
        sys.modules["antenv.axon_hooks"] = mod
        antenv.axon_hooks = mod
        boot_dir = "/root/.axon_site/trn_agent_boot"
        so_path = "/opt/axon/libaxon_pjrt.so"
        if os.path.isdir(boot_dir) and os.path.exists(so_path):
            if boot_dir not in sys.path:
                sys.path.insert(0, boot_dir)
            from trn_boot import _ntff_profile_via_ctypes

            holder["hook"] = _ntff_profile_via_ctypes(so_path)
    except Exception:
        pass


def _build_program():
    """Bass program for ONE core (one channel). SPMD across 8 cores."""
    import concourse.bacc as bacc
    import concourse.mybir as mybir
    from concourse.tile import TileContext

    f32 = mybir.dt.float32
    f32r = mybir.dt.float32r
    # fp32r = fp32 rounded to an 11-bit mantissa (low 12 bits zero), runs the
    # PE at 4x the fp32 rate. The BIR verifier requires every matmul operand's
    # producer to emit float32r-typed output, so the dtype is threaded through
    # DRAM params and SBUF tiles; host pre-rounds the values to the f32r grid.
    main_dt = f32r if MAIN_F32R else f32
    bsel_dt = f32r if BSEL_F32R else f32

    bf16 = mybir.dt.bfloat16

    nc = bacc.Bacc()
    xt_d = nc.declare_dram_parameter(
        "xt", [128, NBH * NKC * 128], main_dt, isOutput=False
    )
    urep_d = nc.declare_dram_parameter("urep", [128, CI], bsel_dt, isOutput=False)
    out_d = nc.declare_dram_parameter("out", [B, CI], f32, isOutput=True)

    with TileContext(nc) as tc:
        with (
            tc.tile_pool(name="const", bufs=1) as const_pool,
            tc.tile_pool(name="w", bufs=NKC) as w_pool,
            tc.tile_pool(name="ob", bufs=2) as o_pool,
            tc.tile_pool(name="ubc", bufs=4, space="PSUM") as ubc_pool,
            tc.tile_pool(name="acc", bufs=4, space="PSUM") as acc_pool,
        ):
            # --- PE warmup: ~5us of dummy matmuls during the DMA wait trips
            # the HAM clock gate to 8/8 so the real matmuls run at 2.4 GHz.
            # Data must NOT be all-zero/all-one (zero-skip would idle the PE).
            warm = const_pool.tile([128, 512], bf16, name="warm")
            nc.gpsimd.iota(
                warm[:], [[1, 512]], base=0, channel_multiplier=3,
                allow_small_or_imprecise_dtypes=True,
            )
            warm_ps = acc_pool.tile([128, 512], f32, tag="acc", name="warm_ps")
            for i in range(14):
                nc.tensor.matmul(
                    warm_ps[:], lhsT=warm[:, 0:128], rhs=warm[:], start=True, stop=True
                )

            urep = const_pool.tile([128, CI], bsel_dt, name="urep")
            nc.sync.dma_start(urep[:], urep_d[:])
            xt = const_pool.tile([128, NBH * NKC * 128], main_dt, name="xt")
            for bh in range(NBH):
                s = bh * NKC * 128
                nc.sync.dma_start(xt[:, s : s + NKC * 128], xt_d[:, s : s + NKC * 128])

            # --- synthesize bsel on device (gpsimd iota + DVE compare):
            # bsel[32*(kc%4)+j, kc*128+p] = 1 iff j == 4*kc + p//32
            # row target per column: F(col) = 36*kc + p//32 - 128*(kc >= 4)
            tcol = const_pool.tile([128, NKC * 128], f32, name="tcol")
            rrow = const_pool.tile([128, 1], f32, name="rrow")
            nc.gpsimd.iota(
                rrow[:], [[0, 1]], base=0, channel_multiplier=1,
                allow_small_or_imprecise_dtypes=True,
            )
            nc.gpsimd.iota(
                tcol[:, 0:512], [[36, 4], [1, 4], [0, 32]], base=0,
                channel_multiplier=0, allow_small_or_imprecise_dtypes=True,
            )
            nc.gpsimd.iota(
                tcol[:, 512:1024], [[36, 4], [1, 4], [0, 32]], base=16,
                channel_multiplier=0, allow_small_or_imprecise_dtypes=True,
            )
            bsel = const_pool.tile([128, NKC * 128], bsel_dt, name="bsel")
            nc.vector.tensor_tensor(
                bsel[:], tcol[:], rrow[:].to_broadcast((128, NKC * 128)),
                op=mybir.AluOpType.is_equal,
            )

            # --- build W chunks: W[kc][(j,k) in chunk, (c,i)] ---
            wts = []
            for kc in range(NKC):
                r = kc % 4
                wt = w_pool.tile([128, CI], main_dt, tag="wt", name=f"wt_{kc}")
                for nh in range(CI // 512):
                    sl = slice(nh * 512, (nh + 1) * 512)
                    ubc = ubc_pool.tile(
                        [128, 512], f32, tag="ubc", name=f"ubc_{kc}_{nh}"
                    )
                    nc.tensor.matmul(
                        ubc[:],
                        lhsT=bsel[32 * r : 32 * r + 32, kc * 128 : (kc + 1) * 128],
                        rhs=urep[32 * r : 32 * r + 32, sl],
                        start=True,
                        stop=True,
                        tile_position=(32 * r, 0),
                    )
                    nc.vector.tensor_mul(wt[:, sl], ubc[:], urep[:, sl].bitcast(f32))
                wts.append(wt)

            # --- main contraction: out[b, ci] = sum_kc xt_kc^T @ W_kc ---
            for bh in range(NBH):
                accs = [
                    acc_pool.tile([128, 512], f32, tag="acc", name=f"acc_{bh}_{i}")
                    for i in range(2)
                ]
                for kc in range(NKC):
                    lhs = xt[
                        :, bh * NKC * 128 + kc * 128 : bh * NKC * 128 + (kc + 1) * 128
                    ]
                    for nh in range(2):
                        nc.tensor.matmul(
                            accs[nh][:],
                            lhsT=lhs,
                            rhs=wts[kc][:, nh * 512 : (nh + 1) * 512],
                            start=(kc == 0),
                            stop=(kc == NKC - 1),
                        )
                ob = o_pool.tile([128, CI], f32, tag="ob", name=f"ob_{bh}")
                for nh in range(2):
                    nc.scalar.copy(ob[:, nh * 512 : (nh + 1) * 512], accs[nh][:])
                nc.sync.dma_start(out_d[bh * 128 : (bh + 1) * 128, :], ob[:])

    nc.finalize()
    return nc


def _build_program_sym():
    """v5: host-built SYMMETRIC W (528 (j<=k) pairs, off-diagonal scaled by
    2), raw bass. The device does only the main fp32r matmuls + output
    copies. DMA chunks alternate between the two HWDGE queues (~180 GB/s
    each) so matmul groups unlock progressively; only the 528 real pairs are
    shipped (chunk 4 is a K=16 matmul)."""
    import concourse.bacc as bacc
    import concourse.mybir as mybir
    from contextlib import ExitStack

    f32 = mybir.dt.float32
    f32r = mybir.dt.float32r
    bf16 = mybir.dt.bfloat16
    main_dt = f32r if MAIN_F32R else f32

    NSC = 5  # symmetric chunks: 4x128 + 1x16 pairs

    nc = bacc.Bacc()
    xs_d = nc.declare_dram_parameter("xs", [128, NSC * B], main_dt, isOutput=False)
    ws_d = nc.declare_dram_parameter("ws", [128, NSC * CI], main_dt, isOutput=False)
    out_d = nc.declare_dram_parameter("out", [B, CI], f32, isOutput=True)

    with ExitStack() as ctx:
        e = ctx.enter_context
        xs = e(nc.sbuf_tensor("xs_sb", [128, NSC * B], main_dt))
        ws = e(nc.sbuf_tensor("ws_sb", [128, NSC * CI], main_dt))
        ob = [e(nc.sbuf_tensor(f"ob{i}", [128, CI], f32)) for i in range(2)]
        warm = e(nc.sbuf_tensor("warm", [128, 512], bf16))
        acc = [e(nc.psum_tensor(f"acc{i}", [128, 512], f32)) for i in range(4)]
        wps = e(nc.psum_tensor("wps", [128, 256], f32))

        # DMA sems: xs main, w0..w2, w3 split in halves, w4+xs4 tail
        sxm = e(nc.semaphore("sxm"))
        w0 = e(nc.semaphore("w0"))
        w1 = e(nc.semaphore("w1"))
        w2 = e(nc.semaphore("w2"))
        w3a = e(nc.semaphore("w3a"))
        w3b = e(nc.semaphore("w3b"))
        w4s = e(nc.semaphore("w4s"))
        sx4 = e(nc.semaphore("sx4"))
        warm_sem = e(nc.semaphore("warm_sem"))
        pe_main = e(nc.semaphore("pe_main"))
        cp_s = e(nc.semaphore("cp_s"))
        cp_v = e(nc.semaphore("cp_v"))
        do0 = e(nc.semaphore("do0"))
        do1 = e(nc.semaphore("do1"))

        block = e(nc.Block())

        # pe_main increment order: acc0, acc2, acc1, acc3
        @block.sync
        def _(sync):
            sync.dma_start(out=xs[:, 0:1024], in_=xs_d[:, 0:1024]).then_inc(sxm, 16)
            sync.dma_start(out=ws[:, 1024:2048], in_=ws_d[:, 1024:2048]).then_inc(
                w1, 16
            )
            sync.dma_start(out=ws[:, 3072:3584], in_=ws_d[:, 3072:3584]).then_inc(
                w3a, 16
            )
            sync.wait_ge(cp_s, 1)
            sync.dma_start(out=out_d[0:128, 0:512], in_=ob[0][:, 0:512]).then_inc(
                do0, 16
            )
            sync.wait_ge(cp_v, 1)
            sync.dma_start(out=out_d[0:128, 512:1024], in_=ob[0][:, 512:1024]).then_inc(
                do0, 16
            )
            sync.wait_ge(do0, 32)

        @block.gpsimd
        def _(g):
            g.iota(
                warm[:], [[1, 512]], base=0, channel_multiplier=3,
                allow_small_or_imprecise_dtypes=True,
            ).then_inc(warm_sem, 1)

        @block.scalar
        def _(s):
            s.dma_start(out=ws[:, 0:1024], in_=ws_d[:, 0:1024]).then_inc(w0, 16)
            s.dma_start(out=ws[:, 2048:3072], in_=ws_d[:, 2048:3072]).then_inc(w2, 16)
            s.dma_start(out=ws[:, 3584:4096], in_=ws_d[:, 3584:4096]).then_inc(w3b, 16)
            s.dma_start(out=ws[0:16, 4096:5120], in_=ws_d[0:16, 4096:5120]).then_inc(
                w4s, 16
            )
            s.dma_start(out=xs[0:16, 1024:1280], in_=xs_d[0:16, 1024:1280]).then_inc(
                sx4, 16
            )
            s.wait_ge(pe_main, 1)
            s.copy(ob[0][:, 0:512], acc[0][:]).then_inc(cp_s, 1)
            s.wait_ge(pe_main, 2)
            s.copy(ob[1][:, 0:512], acc[2][:]).then_inc(cp_s, 1)
            s.wait_ge(cp_s, 2)  # ACT pipeline: ensure the copy retired
            s.dma_start(out=out_d[128:256, 0:512], in_=ob[1][:, 0:512]).then_inc(
                do1, 16
            )
            s.wait_ge(cp_v, 2)
            s.dma_start(out=out_d[128:256, 512:1024], in_=ob[1][:, 512:1024]).then_inc(
                do1, 16
            )
            s.wait_ge(do1, 32)

        @block.tensor
        def _(t):
            def warm_mm(n):
                for _ in range(n):
                    t.matmul(
                        wps[:],
                        lhsT=warm[:, 0:128],
                        rhs=warm[:, 0:256],
                        start=True,
                        stop=True,
                    )

            def group(kc, nh_list=(0, 1), inc=False):
                hi = 16 if kc == 4 else 128
                for nh in nh_list:
                    for bh in range(2):
                        mm = t.matmul(
                            acc[2 * bh + nh][:],
                            lhsT=xs[0:hi, kc * 256 + bh * 128 : kc * 256 + bh * 128 + 128],
                            rhs=ws[
                                0:hi, kc * 1024 + nh * 512 : kc * 1024 + nh * 512 + 512
                            ],
                            start=(kc == 0),
                            stop=(kc == NSC - 1),
                        )
                        if kc == NSC - 1:
                            mm.then_inc(pe_main, 1)

            t.wait_ge(warm_sem, 1)
            warm_mm(10)
            t.wait_ge(sxm, 16)
            t.wait_ge(w0, 16)
            group(0)
            warm_mm(2)  # keep the HAM activity window busy across DMA waits
            t.wait_ge(w1, 16)
            group(1)
            warm_mm(2)
            t.wait_ge(w2, 16)
            group(2)
            t.wait_ge(w3a, 16)
            group(3, nh_list=(0,))
            t.wait_ge(w3b, 16)
            group(3, nh_list=(1,))
            t.wait_ge(w4s, 16)
            t.wait_ge(sx4, 16)
            group(4)

        @block.vector
        def _(v):
            v.wait_ge(pe_main, 3)
            v.tensor_copy(ob[0][:, 512:1024], acc[1][:]).then_inc(cp_v, 1)
            v.wait_ge(pe_main, 4)
            v.tensor_copy(ob[1][:, 512:1024], acc[3][:]).then_inc(cp_v, 1)

    nc.finalize()
    return nc


def _build_program_raw():
    """Hand-scheduled raw-bass version: per-engine streams + manual
    semaphores. Avoids the Tile framework's preamble/drain barriers
    (~10us of fixed overhead) and its conservative pacing."""
    import concourse.bacc as bacc
    import concourse.mybir as mybir
    from contextlib import ExitStack

    f32 = mybir.dt.float32
    f32r = mybir.dt.float32r
    bf16 = mybir.dt.bfloat16
    main_dt = f32r if MAIN_F32R else f32
    bsel_dt = f32r if BSEL_F32R else f32

    nc = bacc.Bacc()
    xt_d = nc.declare_dram_parameter("xt", [128, 2048], main_dt, isOutput=False)
    # ub: urep in cols 0:1024, bsel in cols 1024:2048
    ub_d = nc.declare_dram_parameter("ub", [128, 2048], bsel_dt, isOutput=False)
    out_d = nc.declare_dram_parameter("out", [B, CI], f32, isOutput=True)

    with ExitStack() as ctx:
        e = ctx.enter_context
        xt = e(nc.sbuf_tensor([128, 2048], main_dt))
        ub = e(nc.sbuf_tensor([128, 2048], bsel_dt))
        wt = e(nc.sbuf_tensor([128, 8192], main_dt))  # wt[:, kc*1024+nh*512 ...]
        ob = [e(nc.sbuf_tensor(f"ob{i}", [128, CI], f32)) for i in range(2)]
        warm = e(nc.sbuf_tensor([128, 512], bf16))
        ubc = [e(nc.psum_tensor(f"ubc{i}", [128, 512], f32)) for i in range(4)]
        acc = [e(nc.psum_tensor(f"acc{i}", [128, 512], f32)) for i in range(4)]

        dma_sp = e(nc.semaphore("dma_sp"))
        dma_u = e(nc.semaphore("dma_u"))
        dma_ba = e(nc.semaphore("dma_ba"))
        dma_bb = e(nc.semaphore("dma_bb"))
        dma_act = e(nc.semaphore("dma_act"))
        warm_sem = e(nc.semaphore("warm_sem"))
        pe_sem = e(nc.semaphore("pe_sem"))
        dve_sem = e(nc.semaphore("dve_sem"))
        pe_main = e(nc.semaphore("pe_main"))
        cp_s = e(nc.semaphore("cp_s"))
        cp_v = e(nc.semaphore("cp_v"))

        block = e(nc.Block())

        # pe_main increment order: acc0, acc2, acc1, acc3
        @block.sync
        def _(sync):
            sync.dma_start(out=ub[:, 0:1024], in_=ub_d[:, 0:1024]).then_inc(dma_u, 16)
            sync.dma_start(out=ub[:, 1024:1536], in_=ub_d[:, 1024:1536]).then_inc(
                dma_ba, 16
            )
            sync.dma_start(out=ub[:, 1536:2048], in_=ub_d[:, 1536:2048]).then_inc(
                dma_bb, 16
            )
            sync.wait_ge(cp_s, 1)
            sync.wait_ge(cp_v, 1)
            sync.dma_start(out=out_d[0:128, :], in_=ob[0][:]).then_inc(dma_sp, 16)
            sync.wait_ge(dma_sp, 16)
            sync.wait_ge(dma_u, 16)
            sync.wait_ge(dma_ba, 16)
            sync.wait_ge(dma_bb, 16)

        @block.gpsimd
        def _(g):
            g.iota(
                warm[:], [[1, 512]], base=0, channel_multiplier=3,
                allow_small_or_imprecise_dtypes=True,
            ).then_inc(warm_sem, 1)

        @block.scalar
        def _(s):
            s.dma_start(out=xt[:, 0:1024], in_=xt_d[:, 0:1024]).then_inc(dma_act, 16)
            s.dma_start(out=xt[:, 1024:2048], in_=xt_d[:, 1024:2048]).then_inc(
                dma_act, 16
            )
            # output copies for nh=0 halves (nh=1 goes to DVE)
            s.wait_ge(pe_main, 1)
            s.copy(ob[0][:, 0:512], acc[0][:]).then_inc(cp_s, 1)
            s.wait_ge(pe_main, 2)
            s.copy(ob[1][:, 0:512], acc[2][:]).then_inc(cp_s, 1)
            # second output DMA rides the ACT HWDGE queue, parallel to sync's
            s.wait_ge(cp_v, 2)
            s.dma_start(out=out_d[128:256, :], in_=ob[1][:]).then_inc(dma_act, 16)
            s.wait_ge(dma_act, 48)

        @block.tensor
        def _(t):
            t.wait_ge(warm_sem, 1)
            for i in range(6):
                t.matmul(
                    acc[0][:], lhsT=warm[:, 0:128], rhs=warm[:], start=True, stop=True
                )

            def bsel_mm(i):
                kc, nh = i // 2, i % 2
                r = kc % 4
                if i >= 4:
                    t.wait_ge(dve_sem, i - 3)  # ubc buffer recycle (4 bufs)
                t.matmul(
                    ubc[i % 4][:],
                    lhsT=ub[
                        32 * r : 32 * r + 32, 1024 + kc * 128 : 1024 + kc * 128 + 128
                    ],
                    rhs=ub[32 * r : 32 * r + 32, nh * 512 : nh * 512 + 512],
                    start=True,
                    stop=True,
                    tile_position=(32 * r, 0),
                ).then_inc(pe_sem, 1)

            def main_group(kc):
                # TTs 2kc,2kc+1 already guaranteed by bsel_mm(2kc+5)'s wait,
                # except for the trailing kc groups which wait explicitly.
                for nh in range(2):
                    for bh in range(2):
                        mm = t.matmul(
                            acc[2 * bh + nh][:],
                            lhsT=xt[
                                :, bh * 1024 + kc * 128 : bh * 1024 + kc * 128 + 128
                            ],
                            rhs=wt[
                                :, kc * 1024 + nh * 512 : kc * 1024 + nh * 512 + 512
                            ],
                            start=(kc == 0),
                            stop=(kc == NKC - 1),
                        )
                        if kc == NKC - 1:
                            mm.then_inc(pe_main, 1)

            # interleave selection matmuls with main matmuls as W halves land
            t.wait_ge(dma_u, 16)
            t.wait_ge(dma_ba, 16)  # urep + bsel(kc 0..3)
            for i in range(4):
                bsel_mm(i)
            t.wait_ge(dma_act, 32)  # xt fully resident
            bsel_mm(4)
            bsel_mm(5)
            main_group(0)
            bsel_mm(6)
            bsel_mm(7)
            main_group(1)
            t.wait_ge(dma_bb, 16)  # bsel(kc 4..7)
            for kc in range(2, 6):
                bsel_mm(2 * kc + 4)
                bsel_mm(2 * kc + 5)
                main_group(kc)
            t.wait_ge(dve_sem, 14)
            main_group(6)
            t.wait_ge(dve_sem, 16)
            main_group(7)

        @block.vector
        def _(v):
            v.wait_ge(dma_u, 16)
            for i in range(16):
                kc, nh = i // 2, i % 2
                v.wait_ge(pe_sem, i + 1)
                v.tensor_mul(
                    wt[:, kc * 1024 + nh * 512 : kc * 1024 + nh * 512 + 512],
                    ubc[i % 4][:],
                    ub[:, nh * 512 : nh * 512 + 512].bitcast(f32),
                ).then_inc(dve_sem, 1)
            # output copies for nh=1 halves
            for bh in range(2):
                v.wait_ge(pe_main, bh + 3)  # acc1 done at 3, acc3 at 4
                v.tensor_copy(ob[bh][:, 512:1024], acc[2 * bh + 1][:]).then_inc(
                    cp_v, 1
                )

    nc.finalize()
    return nc


def _get_program():
    global _PROGRAM
    if _PROGRAM is None:
        _import_concourse()
        _PROGRAM = {
            "sym": _build_program_sym,
            "raw": _build_program_raw,
            "tile": _build_program,
        }[MODE]()
    return _PROGRAM


def _host_prep_sym(X, C):
    """Symmetric-W host prep: pack the (j<=k) triangle of X and the scaled
    eigenvector-product matrix W; 528 pairs padded to 640 (pad rows of W are
    zero, so they contribute nothing)."""
    X = np.ascontiguousarray(np.asarray(X, dtype=np.float32))
    V = _eigvecs(np.asarray(C, dtype=np.float32))  # [c, n, j, i]
    U = V.transpose(1, 2, 0, 3).reshape(N_CH, P, CI)  # [n, k, ci]

    jj, kk = np.triu_indices(P)  # 528 pairs
    scale = np.where(jj == kk, 1.0, 2.0).astype(np.float32)[None, :, None]
    W = U[:, jj, :] * U[:, kk, :] * scale  # [n, 528, ci]
    Wp = np.zeros((N_CH, 640, CI), np.float32)
    Wp[:, :528] = W
    ws = Wp.reshape(N_CH, 5, 128, CI).transpose(0, 2, 1, 3).reshape(N_CH, 128, 5 * CI)

    Xs = X[:, :, jj, kk].transpose(1, 2, 0)  # [n, 528, b]
    Xsp = np.zeros((N_CH, 640, B), np.float32)
    Xsp[:, :528] = Xs
    # kc-major layout: xs[p, kc*256 + bh*128 + bb]
    xs = (
        Xsp.reshape(N_CH, 5, 128, NBH, 128)
        .transpose(0, 2, 1, 3, 4)
        .reshape(N_CH, 128, 5 * NBH * 128)
    )
    if MAIN_F32R:
        ws = _round_fp32r(ws)
        xs = _round_fp32r(xs)
    return np.ascontiguousarray(xs), np.ascontiguousarray(ws)


def _eigvecs(C):
    # jax CPU eigh reproduces the reference's eigenvectors bit-for-bit;
    # a from-scratch f64 eigh would differ by the reference's own f32 eigh
    # error (~3e-4 in the output) on near-degenerate eigenpairs.
    try:
        import jax
        import jax.numpy as jnp

        with jax.default_device(jax.devices("cpu")[0]):
            _, V = jnp.linalg.eigh(jnp.asarray(C, dtype=jnp.float32))
            return np.asarray(V)
    except Exception:
        _, V = np.linalg.eigh(C.astype(np.float64))
        return V.astype(np.float32)


def _round_fp32r(a):
    """Round to the fp32r grid (11-bit mantissa, RNE), matching the PE's
    fp32_to_fp32r downconversion. Idempotent; exact on 0/1."""
    u = np.ascontiguousarray(a, dtype=np.float32).view(np.uint32)
    lsb = (u >> 12) & np.uint32(1)
    r = u + np.uint32(0x7FF) + lsb
    return (r & np.uint32(0xFFFFF000)).view(np.float32)


def _host_prep(X, C):
    """Host-side: eigh + per-core input layouts."""
    X = np.ascontiguousarray(np.asarray(X, dtype=np.float32))
    C = np.asarray(C, dtype=np.float32)

    V = _eigvecs(C)  # [NC, N_CH, P(j), P(i)]
    if MAIN_F32R:
        X = _round_fp32r(X)
    if BSEL_F32R:
        V = _round_fp32r(V)

    # U[n][k, c*P+i] = V[c, n, k, i]
    U = V.transpose(1, 2, 0, 3).reshape(N_CH, P, CI)
    urep = np.ascontiguousarray(np.tile(U, (1, 4, 1)))  # [n, 128, CI]

    # xt[n][p, bh*1024 + kc*128 + bb] = X[bh*128+bb, n, j, k], jk = kc*128+p
    Xt = X.transpose(1, 2, 3, 0).reshape(N_CH, NKC, 128, NBH, 128)
    xt = np.ascontiguousarray(
        Xt.transpose(0, 2, 3, 1, 4).reshape(N_CH, 128, NBH * NKC * 128)
    )

    # bsel[32*(kc%4) + j, kc*128 + p] = 1 iff j == 4*kc + p//32 (raw path
    # DMAs this with urep; the Tile path synthesizes it on device)
    bsel = np.zeros((128, NKC * 128), np.float32)
    for kc in range(NKC):
        r = kc % 4
        for p in range(128):
            bsel[32 * r + 4 * kc + p // 32, kc * 128 + p] = 1.0
    return xt, urep, bsel


def _reassemble(outs):
    # outs: list of 8 arrays [B, CI]; diag[b, c, n, i] = outs[n][b, c*P+i]
    full = np.stack(outs, axis=0).reshape(N_CH, B, NC, P)
    return np.ascontiguousarray(full.transpose(1, 2, 0, 3))


LAST_RESULTS = None  # BassKernelResults from the most recent device run


def kernel(X, C, idx=None, **_unused):
    global LAST_RESULTS
    _import_concourse()

    nc = _get_program()
    if MODE == "sym":
        xs, ws = _host_prep_sym(X, C)
        in_maps = [{"xs": xs[n], "ws": ws[n]} for n in range(N_CH)]
    elif MODE == "raw":
        xt, urep, bsel = _host_prep(X, C)
        ub = np.concatenate(
            [urep, np.broadcast_to(bsel, (N_CH, 128, NKC * 128))], axis=2
        )
        in_maps = [
            {"xt": xt[n], "ub": np.ascontiguousarray(ub[n])} for n in range(N_CH)
        ]
    else:
        xt, urep, bsel = _host_prep(X, C)
        in_maps = [{"xt": xt[n], "urep": urep[n]} for n in range(N_CH)]

    if os.environ.get("KERNEL_SIM", "0") == "1":
        from concourse import bass_interp

        sim = bass_interp.MultiCoreSim(nc, N_CH)
        for n in range(N_CH):
            for name, arr in in_maps[n].items():
                sim.cores[n].tensor(name)[:] = arr
        sim.simulate()
        outs = [np.array(sim.cores[n].mem_tensor("out")) for n in range(N_CH)]
    else:
        from concourse import bass_utils

        res = bass_utils.run_bass_kernel_spmd(
            nc,
            in_maps,
            list(range(N_CH)),
            trace=os.environ.get("KERNEL_TRACE", "0") == "1",
        )
        LAST_RESULTS = res
        outs = [res.results[n]["out"] for n in range(N_CH)]

    return _reassemble(outs)
